# revision 1
# baseline (speedup 1.0000x reference)
"""Trainium2 Bass kernel for LoopConnectivityDecoder.

Math: out[i,j] (i<j) = sigmoid( sum_k W2[k] * relu(a'[i,k] + b'[k,j]) + b2 ),
symmetrized, zero diagonal; a' = X@W1[:,:32].T + b1, b' = (X@W1[:,32:].T).T.

Device strategy (8 cores, SPMD, per-core work fixed by host-side gathers):
- Signed scale folded into data: z_k = W2[k]*a' + W2[k]*b'. Then
  W2[k]*relu(a'+b') = max(z,0) if W2[k]>=0 else min(z,0).
- Upper triangle covered by 24 uniform (128 x 512) units, 3 per core.
- Per k: one K=4 bf16 matmul computes the outer sum z in PSUM at full fp32
  accuracy via hi/lo bf16 splitting: lhsT=[a_hi;a_lo;1;1], rhs=[1;1;b_hi;b_lo].
- k's are sign-grouped and chunked by 4 (groups zero-padded to 4-multiples):
  4 matmuls fill a (128,4,512) PSUM tile; ScalarE drains it with one fused
  relu (scale=+/-1 by sign) into SBUF; VectorE/GpSimd run 4-wide interleaved
  accumulate chains (scalar_tensor_tensor: acc = staged*(+/-1) + acc).
- Tail per unit: merge chains, sigmoid(+b2) on ScalarE, DMA out.
- Host scatters unit tiles into the full matrix, applies triu, mirrors.
"""

import numpy as np
import ml_dtypes

N = 1536
EMB = 32
H = 64
P = 128          # partition tile (rows per unit)
F = 512          # free-dim tile (cols per unit)
NCORES = 8
NBLK = N // P    # 12 row blocks
UNITS_PER_CORE = 3
CH = 4           # k's per chunk (PSUM tile = CH banks; build-time override)
LDG = 8          # k-slots per DMA load group

_cache = {}


def _unit_list():
    """24 (row_block, col0) units covering the upper-triangle staircase."""
    units = []
    for bi in range(NBLK):
        cols = N - P * bi
        nch = -(-cols // F)
        for t in range(nch):
            col0 = min(P * bi + F * t, N - F)
            units.append((bi, col0))
    assert len(units) == NCORES * UNITS_PER_CORE
    return units


def _slot_list(pos_mask, ch=CH):
    """Sign-grouped, zero-padded slot list.

    Returns (slots, chunk_signs): slots[i] is a k index or None (zero pad);
    chunk_signs[c] is +1/-1 for slots[ch*c : ch*(c+1)]."""
    pos = [k for k in range(H) if pos_mask[k]]
    neg = [k for k in range(H) if not pos_mask[k]]
    slots, signs = [], []
    for grp, sgn in ((pos, 1.0), (neg, -1.0)):
        if not grp:
            continue
        pad = (-len(grp)) % ch
        g = [None] * pad + grp
        slots += g
        signs += [sgn] * (len(g) // ch)
    assert len(slots) % ch == 0
    return slots, signs


def _build_module(pos_mask, repeat=1, n_dve_chunks=None, ablate="full",
                  stg_bufs=4, psum_bufs=2, stage_bf16=False, ch=CH):
    """Build + compile the Bass module. pos_mask: tuple of 64 bools."""
    from contextlib import ExitStack
    import concourse.tile as tile
    from concourse import bacc, mybir

    slots, signs = _slot_list(pos_mask, ch)
    S = len(slots)
    NCH = S // ch
    NLD = -(-S // LDG)
    if n_dve_chunks is None:
        n_dve_chunks = max(1, min(NCH - 1, round(NCH * 11 / 17)))
    if ablate == "nopool":
        n_dve_chunks = NCH

    nc = bacc.Bacc("TRN2", target_bir_lowering=False, debug=False,
                   num_devices=NCORES)
    A1_d = nc.dram_tensor("A1g", [4, S, UNITS_PER_CORE * P], mybir.dt.bfloat16,
                          kind="ExternalInput")
    B1_d = nc.dram_tensor("B1g", [4, S, UNITS_PER_CORE * F], mybir.dt.bfloat16,
                          kind="ExternalInput")
    b2_d = nc.dram_tensor("b2c", [P, 1], mybir.dt.float32, kind="ExternalInput")
    out_d = nc.dram_tensor("out", [UNITS_PER_CORE, P, F], mybir.dt.float32,
                           kind="ExternalOutput")

    with tile.TileContext(nc) as tc, ExitStack() as ctx:
        const = ctx.enter_context(tc.tile_pool(name="const", bufs=1))
        ld = ctx.enter_context(tc.tile_pool(name="ld", bufs=4))
        stg = ctx.enter_context(tc.tile_pool(name="stg", bufs=stg_bufs))
        accp = ctx.enter_context(tc.tile_pool(name="accp", bufs=2))
        outp = ctx.enter_context(tc.tile_pool(name="outp", bufs=2))
        psum = ctx.enter_context(tc.tile_pool(name="psum", bufs=psum_bufs, space="PSUM"))

        b2_t = const.tile([P, 1], mybir.dt.float32)
        nc.sync.dma_start(b2_t[:], b2_d[:])

        def body():
            for u in range(UNITS_PER_CORE):
                a_tiles, b_tiles = [], []
                for g in range(NLD):
                    s0 = g * LDG
                    sw = min(LDG, S - s0)
                    a_t = ld.tile([4, LDG, P], mybir.dt.bfloat16, tag="a")
                    nc.sync.dma_start(
                        a_t[:, 0:sw], A1_d[:, s0:s0 + sw, u * P:(u + 1) * P])
                    b_t = ld.tile([4, LDG, F], mybir.dt.bfloat16, tag="b")
                    nc.sync.dma_start(
                        b_t[:, 0:sw], B1_d[:, s0:s0 + sw, u * F:(u + 1) * F])
                    a_tiles.append(a_t)
                    b_tiles.append(b_t)

                accD = accN = None
                for c in range(NCH):
                    sgn = signs[c]
                    y = psum.tile([P, ch, F], mybir.dt.float32, tag="y")
                    for q in range(ch):
                        s = c * ch + q
                        g, off = s // LDG, s % LDG
                        nc.tensor.matmul(y[:, q],
                                         a_tiles[g][0:4, off, :],
                                         b_tiles[g][0:4, off, :],
                                         start=True, stop=True)
                    sdt = mybir.dt.bfloat16 if stage_bf16 else mybir.dt.float32
                    t4 = stg.tile([P, ch, F], sdt, tag="t4")
                    nc.scalar.activation(t4[:], y[:],
                                         mybir.ActivationFunctionType.Relu,
                                         scale=float(sgn))
                    if ablate == "noacc":
                        if c == NCH - 1:
                            accD = t4
                        continue
                    # accumulate: acc += sgn * t4 (4-wide interleaved chain)
                    on_dve = c < n_dve_chunks
                    if on_dve:
                        newacc = accp.tile([P, ch, F], mybir.dt.float32,
                                           tag="accD")
                        if accD is None:
                            nc.vector.tensor_scalar(newacc[:], t4[:],
                                                    float(sgn), None,
                                                    mybir.AluOpType.mult)
                        else:
                            nc.vector.scalar_tensor_tensor(
                                newacc[:], t4[:], float(sgn), accD[:],
                                mybir.AluOpType.mult, mybir.AluOpType.add)
                        accD = newacc
                    else:
                        # gpsimd: walrus rejects TensorScalarPtr on Pool, so
                        # chain with plain tensor_tensor add/subtract.
                        newacc = accp.tile([P, ch, F], mybir.dt.float32,
                                           tag="accN")
                        if accN is None:
                            accN = accp.tile([P, ch, F], mybir.dt.float32,
                                             tag="accN")
                            nc.gpsimd.memset(accN[:], 0.0)
                        op = (mybir.AluOpType.add if sgn > 0
                              else mybir.AluOpType.subtract)
                        nc.gpsimd.tensor_tensor(newacc[:], accN[:], t4[:], op)
                        accN = newacc

                # merge chains: logit = sum over ch slices (+ gpsimd chain)
                lg = outp.tile([P, F], mybir.dt.float32, tag="lg")
                def fold(eng, acc):
                    w = ch
                    while w > 1:
                        half = w // 2
                        nxt = outp.tile([P, half, F], mybir.dt.float32,
                                        tag=f"fold{half}")
                        eng.tensor_tensor(nxt[:], acc[:, 0:half],
                                          acc[:, half:2 * half],
                                          mybir.AluOpType.add)
                        acc, w = nxt, half
                    return acc
                aD = fold(nc.vector, accD)
                if accN is not None and ablate != "noacc":
                    aN = fold(nc.gpsimd, accN)
                    nc.vector.tensor_tensor(lg[:], aD[:, 0], aN[:, 0],
                                            mybir.AluOpType.add)
                else:
                    nc.vector.tensor_copy(lg[:], aD[:, 0])
                s_t = outp.tile([P, F], mybir.dt.float32, tag="s")
                nc.scalar.activation(s_t[:], lg[:],
                                     mybir.ActivationFunctionType.Sigmoid,
                                     bias=b2_t[:, 0:1], scale=1.0)
                nc.sync.dma_start(out_d[u], s_t[:])

        if repeat > 1:
            with tc.For_i(0, repeat, 1):
                body()
        else:
            body()

    nc.compile()
    return nc


def _split_bf16(x):
    """Split fp32 array into (hi, lo) bf16 arrays with hi+lo ~= x."""
    hi = x.astype(ml_dtypes.bfloat16)
    lo = (x - hi.astype(np.float32)).astype(ml_dtypes.bfloat16)
    return hi, lo


def _prep_inputs(loop_embeddings, W1, b1, W2, b2):
    X = np.asarray(loop_embeddings, dtype=np.float32)
    W1 = np.asarray(W1, dtype=np.float32)
    b1 = np.asarray(b1, dtype=np.float32)
    W2 = np.asarray(W2, dtype=np.float32)
    b2 = np.asarray(b2, dtype=np.float32)

    a = X @ W1[:, :EMB].T + b1          # (N, H)
    bm = X @ W1[:, EMB:].T              # (N, H)
    w2 = W2[0]

    az = (w2[None, :] * a).T            # (H, N): z-contribution rows (i)
    bz = (w2[None, :] * bm).T           # (H, N): z-contribution rows (j)
    az_hi, az_lo = _split_bf16(az)
    bz_hi, bz_lo = _split_bf16(bz)

    pos_mask = tuple(bool(v) for v in (w2 >= 0))
    slots, _ = _slot_list(pos_mask)
    S = len(slots)
    units = _unit_list()

    in_maps = []
    for core in range(NCORES):
        A1g = np.zeros((4, S, UNITS_PER_CORE * P), dtype=ml_dtypes.bfloat16)
        B1g = np.zeros((4, S, UNITS_PER_CORE * F), dtype=ml_dtypes.bfloat16)
        for u in range(UNITS_PER_CORE):
            bi, col0 = units[core * UNITS_PER_CORE + u]
            for s, k in enumerate(slots):
                if k is None:
                    continue
                A1g[0, s, u * P:(u + 1) * P] = az_hi[k, bi * P:(bi + 1) * P]
                A1g[1, s, u * P:(u + 1) * P] = az_lo[k, bi * P:(bi + 1) * P]
                A1g[2, s, u * P:(u + 1) * P] = 1.0
                A1g[3, s, u * P:(u + 1) * P] = 1.0
                B1g[0, s, u * F:(u + 1) * F] = 1.0
                B1g[1, s, u * F:(u + 1) * F] = 1.0
                B1g[2, s, u * F:(u + 1) * F] = bz_hi[k, col0:col0 + F]
                B1g[3, s, u * F:(u + 1) * F] = bz_lo[k, col0:col0 + F]
        in_maps.append({
            "A1g": A1g,
            "B1g": B1g,
            "b2c": np.full((P, 1), b2[0], dtype=np.float32),
        })
    return in_maps, pos_mask, units


def kernel(loop_embeddings, W1, b1, W2, b2):
    from concourse.bass_utils import run_bass_kernel_spmd

    in_maps, pos_mask, units = _prep_inputs(loop_embeddings, W1, b1, W2, b2)

    if pos_mask not in _cache:
        _cache[pos_mask] = _build_module(pos_mask)
    nc = _cache[pos_mask]

    res = run_bass_kernel_spmd(nc, in_maps, list(range(NCORES)))

    s = np.zeros((N, N), dtype=np.float32)
    for core in range(NCORES):
        o = res.results[core]["out"]
        for u in range(UNITS_PER_CORE):
            bi, col0 = units[core * UNITS_PER_CORE + u]
            s[bi * P:(bi + 1) * P, col0:col0 + F] = o[u]
    up = np.triu(s, 1)
    return (up + up.T).astype(np.float32)



# revision 3
# speedup vs baseline: 5313.3608x; 5313.3608x over previous
"""Trainium2 Bass kernel v2 for LoopConnectivityDecoder.

Math: out[i,j] (i<j) = sigmoid( sum_k w2_k * relu(a[i,k] + b[j,k]) + b2 ),
symmetrized, zero diagonal; a = X@W1[:, :32].T + b1, b = X@W1[:, 32:].T.

Per (i,j) tile the kernel computes z_k = w2_k*(a_ik + b_jk) as a rank-2
outer sum on the tensor engine (K=2 bf16 matmul: lhsT=[a_k;1], rhs=[1;b_k]),
then w2_k*relu(h) = max(z,0) for w2_k>=0 / min(z,0) for w2_k<0.

Device strategy (8 cores SPMD, upper triangle in 24 (128x512) units,
3 per core; 64 k-slots sign-ordered, 16 chunks of 4):
- PE: 4x row-tiled matmuls (tile_position=(32q,0)), slot 4c+q reads its
  [a;1]/[1;b] rows at SBUF partitions 32q+2c -> 4 concurrent MMs/chunk.
- Chunk drains split across engines (relu+accumulate):
  D-path:  DVE scalar_tensor_tensor acc += max/min(z_psum,0), stride-0
           in-place accumulator (one op folds the 4 k-slices).
  S-path:  ScalarE relu(+-z) -> fp16 staged tile, then either DVE
           tensor_tensor fp16 (2x mode) or GpSimd tensor_tensor adds
           into per-path accumulators (also stride-0 in-place).
- Tail: merge 3 accumulators, sigmoid(+b2) on ScalarE, DMA out.
- Host: fold w2/b1 into bf16 a/b row tables, scatter/mirror output.
"""

import numpy as np
import ml_dtypes

N = 1536
EMB = 32
H = 64
P = 128          # rows per unit
F = 512          # cols per unit
NCORES = 8
NBLK = N // P    # 12 row blocks
UNITS_PER_CORE = 3
CH = 4           # k-slots per chunk
NCHUNK = H // CH  # 16

# chunk drain quotas per unit (16 chunks): D = DVE fused stt,
# V = ScalarE stage + deferred DVE fp16 add, G = ScalarE stage + deferred
# GpSimd add.  Adds run at unit end so the PSUM-drain ops (STT on DVE, act
# on ScalarE) hit their engine FIFOs without cross-engine inversions.
QUOTA = {"D": 6, "V": 6, "G": 4}


def _chunk_types(npos):
    """Per-chunk drain type; the mixed-sign chunk (if any) goes to D."""
    mixed = npos // CH if npos % CH else -1
    types = [None] * NCHUNK
    counts = dict(QUOTA)
    if 0 <= mixed < NCHUNK and counts["D"] > 0:
        types[mixed] = "D"
        counts["D"] -= 1
    # largest-remainder interleave of the remaining quotas
    left = dict(counts)
    credit = {k: 0.0 for k in left}
    total = sum(left.values())
    for c in range(NCHUNK):
        if types[c] is not None:
            continue
        for k in left:
            credit[k] += counts[k] / total
        avail = [k for k in left if left[k] > 0]
        best = max(avail, key=lambda k: credit[k])
        types[c] = best
        left[best] -= 1
        credit[best] -= 1.0
    return types

_cache = {}


def _unit_list():
    """24 (row_block, col0) units covering the upper-triangle staircase."""
    units = []
    for bi in range(NBLK):
        cols = N - P * bi
        nch = -(-cols // F)
        for t in range(nch):
            col0 = min(P * bi + F * t, N - F)
            units.append((bi, col0))
    assert len(units) == NCORES * UNITS_PER_CORE
    return units


def _chunk_segments(npos):
    """Per chunk: list of (offset, width, sign) sub-ranges by w2 sign."""
    segs = []
    for c in range(NCHUNK):
        s0, s1 = c * CH, (c + 1) * CH
        if s1 <= npos:
            segs.append([(0, CH, 1.0)])
        elif s0 >= npos:
            segs.append([(0, CH, -1.0)])
        else:
            m = npos - s0
            segs.append([(0, m, 1.0), (m, CH - m, -1.0)])
    return segs


def _build_module(npos, repeat=1, ch=None, psum_bufs=2, scalar_dma=False,
                  defer_last=False, quota=None, lag=7):
    from contextlib import ExitStack
    import concourse.tile as tile
    from concourse import bacc, mybir

    global CH, NCHUNK, QUOTA
    if ch is not None:
        CH = ch
        NCHUNK = H // CH
    if quota is not None:
        QUOTA = dict(quota)
    segs = _chunk_segments(npos)
    types = _chunk_types(npos)

    nc = bacc.Bacc("TRN2", target_bir_lowering=False, debug=False,
                   num_devices=NCORES)
    A_d = nc.dram_tensor("Ag", [UNITS_PER_CORE, CH, 2, NCHUNK, P],
                         mybir.dt.bfloat16, kind="ExternalInput")
    B_d = nc.dram_tensor("Bg", [UNITS_PER_CORE, CH, 2, NCHUNK, F],
                         mybir.dt.bfloat16, kind="ExternalInput")
    b2_d = nc.dram_tensor("b2c", [P, 1], mybir.dt.float32,
                          kind="ExternalInput")
    out_d = nc.dram_tensor("out", [UNITS_PER_CORE, P, F], mybir.dt.float32,
                           kind="ExternalOutput")

    with tile.TileContext(nc) as tc, ExitStack() as ctx:
        const = ctx.enter_context(tc.tile_pool(name="const", bufs=1))
        ld = ctx.enter_context(tc.tile_pool(name="ld", bufs=1))
        stg = ctx.enter_context(tc.tile_pool(name="stg", bufs=12))
        accp = ctx.enter_context(tc.tile_pool(name="accp", bufs=1))
        outp = ctx.enter_context(tc.tile_pool(name="outp", bufs=2))
        psum = ctx.enter_context(tc.tile_pool(name="psum", bufs=psum_bufs,
                                              space="PSUM"))

        b2_t = const.tile([P, 1], mybir.dt.float32)
        nc.sync.dma_start(b2_t[:], b2_d[:])

        def body():
            # interleaved streams: chunk (u, c) for u in 0..2, c in 0..15,
            # emitted u-major within each c so consecutive stream slots hit
            # different PSUM buffers and different drain engines.
            a_ts, b_ts, accs = [], [], []
            for u in range(UNITS_PER_CORE):
                a_t = ld.tile([98, NCHUNK, P], mybir.dt.bfloat16, tag=f"a{u}")
                b_t = ld.tile([98, NCHUNK, F], mybir.dt.bfloat16, tag=f"b{u}")
                for q in range(CH):
                    nc.sync.dma_start(a_t[32 * q:32 * q + 2], A_d[u, q])
                    (nc.scalar if scalar_dma else nc.sync).dma_start(
                        b_t[32 * q:32 * q + 2], B_d[u, q])
                a_ts.append(a_t)
                b_ts.append(b_t)
                accD = accp.tile([P, 1, F], mybir.dt.float32, tag=f"accD{u}")
                accV = accp.tile([P, 1, F], mybir.dt.float16, tag=f"accV{u}")
                accG = accp.tile([P, 1, F], mybir.dt.float32, tag=f"accG{u}")
                nc.vector.memset(accD[:], 0.0)
                nc.vector.memset(accV[:], 0.0)
                nc.gpsimd.memset(accG[:], 0.0)
                accs.append((accD, accV, accG))

            # per-unit type schedule: cyclic shifts so stream neighbours use
            # different drain engines; mixed-sign chunk pinned to D.
            mixed = npos // CH if npos % CH else -1
            types_u = []
            for u in range(UNITS_PER_CORE):
                t = [types[(c + 5 * u) % NCHUNK] for c in range(NCHUNK)]
                if 0 <= mixed < NCHUNK and t[mixed] != "D":
                    j = next(i for i in range(NCHUNK) if t[i] == "D")
                    t[j], t[mixed] = t[mixed], "D"
                types_u.append(t)

            def _do_adds(u, kind, t4, seg):
                _, accV, accG = accs[u]
                for off, w, sgn in seg:
                    op1 = (mybir.AluOpType.add if sgn > 0
                           else mybir.AluOpType.subtract)
                    if kind == "V":
                        bV = accV[:].broadcast_to([P, w, F])
                        nc.vector.tensor_tensor(
                            bV, bV, t4[:, off:off + w], op1)
                    else:
                        bG = accG[:].broadcast_to([P, w, F])
                        nc.gpsimd.tensor_tensor(
                            bG, bG, t4[:, off:off + w], op1)

            LAG = lag
            pending = []  # fifo of (u, kind, t4, seg)
            for c in range(NCHUNK):
                for u in range(UNITS_PER_CORE):
                    kind = types_u[u][c]
                    y = psum.tile([P, CH, F], mybir.dt.float32, tag="y")
                    for q in range(CH):
                        nc.tensor.matmul(y[:, q],
                                         a_ts[u][32 * q:32 * q + 2, c, :],
                                         b_ts[u][32 * q:32 * q + 2, c, :],
                                         start=True, stop=True,
                                         tile_position=(32 * q, 0))
                    if kind == "D":
                        accD = accs[u][0]
                        for off, w, sgn in segs[c]:
                            op0 = (mybir.AluOpType.max if sgn > 0
                                   else mybir.AluOpType.min)
                            bD = accD[:].broadcast_to([P, w, F])
                            nc.vector.scalar_tensor_tensor(
                                bD, y[:, off:off + w], 0.0, bD,
                                op0, mybir.AluOpType.add)
                    else:
                        t4 = stg.tile([P, CH, F], mybir.dt.float16, tag="t4")
                        for off, w, sgn in segs[c]:
                            nc.scalar.activation(
                                t4[:, off:off + w], y[:, off:off + w],
                                mybir.ActivationFunctionType.Relu,
                                scale=float(sgn))
                        pending.append((u, kind, t4, segs[c]))
                    while len(pending) > LAG:
                        _do_adds(*pending.pop(0))

            for item in pending:
                _do_adds(*item)

            for u in range(UNITS_PER_CORE):
                accD, accV, accG = accs[u]
                lg = outp.tile([P, F], mybir.dt.float32, tag="lg")
                nc.gpsimd.tensor_tensor(lg[:], accD[:, 0], accV[:, 0],
                                        mybir.AluOpType.add)
                nc.vector.tensor_tensor(lg[:], lg[:], accG[:, 0],
                                        mybir.AluOpType.add)
                s_t = outp.tile([P, F], mybir.dt.float32, tag="s")
                nc.scalar.activation(s_t[:], lg[:],
                                     mybir.ActivationFunctionType.Sigmoid,
                                     bias=b2_t[:, 0:1], scale=1.0)
                nc.sync.dma_start(out_d[u], s_t[:])

        if repeat > 1:
            with tc.For_i(0, repeat, 1):
                body()
        else:
            body()

    nc.compile()
    return nc


def _prep_inputs(loop_embeddings, W1, b1, W2, b2):
    X = np.asarray(loop_embeddings, dtype=np.float32)
    W1 = np.asarray(W1, dtype=np.float32)
    b1 = np.asarray(b1, dtype=np.float32)
    W2 = np.asarray(W2, dtype=np.float32)
    b2 = np.asarray(b2, dtype=np.float32)

    a = X @ W1[:, :EMB].T + b1          # (N, H)
    bm = X @ W1[:, EMB:].T              # (N, H)
    w2 = W2[0]

    pos = np.where(w2 >= 0)[0]
    neg = np.where(w2 < 0)[0]
    order = np.concatenate([pos, neg])
    npos = len(pos)

    az = (w2[None, :] * a).T[order].astype(ml_dtypes.bfloat16)   # (H, N)
    bz = (w2[None, :] * bm).T[order].astype(ml_dtypes.bfloat16)  # (H, N)

    units = _unit_list()

    in_maps = []
    for core in range(NCORES):
        A = np.ones((UNITS_PER_CORE, CH, 2, NCHUNK, P), dtype=ml_dtypes.bfloat16)
        B = np.ones((UNITS_PER_CORE, CH, 2, NCHUNK, F), dtype=ml_dtypes.bfloat16)
        for u in range(UNITS_PER_CORE):
            bi, col0 = units[core * UNITS_PER_CORE + u]
            # slot s = CH*c + q lives at A[u, q, 0, c] / B[u, q, 1, c]
            A[u, :, 0] = az[:, bi * P:(bi + 1) * P] \
                .reshape(NCHUNK, CH, P).transpose(1, 0, 2)
            B[u, :, 1] = bz[:, col0:col0 + F] \
                .reshape(NCHUNK, CH, F).transpose(1, 0, 2)
        in_maps.append({
            "Ag": A,
            "Bg": B,
            "b2c": np.full((P, 1), b2[0], dtype=np.float32),
        })
    return in_maps, npos, units


TRACE = False
LAST_EXEC_NS = None


def kernel(loop_embeddings, W1, b1, W2, b2):
    from concourse.bass_utils import run_bass_kernel_spmd

    in_maps, npos, units = _prep_inputs(loop_embeddings, W1, b1, W2, b2)

    if npos not in _cache:
        _cache[npos] = _build_module(npos)
    nc = _cache[npos]

    res = run_bass_kernel_spmd(nc, in_maps, list(range(NCORES)))

    s = np.zeros((N, N), dtype=np.float32)
    for core in range(NCORES):
        o = res.results[core]["out"]
        for u in range(UNITS_PER_CORE):
            bi, col0 = units[core * UNITS_PER_CORE + u]
            s[bi * P:(bi + 1) * P, col0:col0 + F] = o[u]
    up = np.triu(s, 1)
    return (up + up.T).astype(np.float32)
